# revision 32
# baseline (speedup 1.0000x reference)
"""Trainium2 kernel for per-row iterative 2-bit affine quantization (ALS).

Self-contained: accepts FULL inputs w[4096,8192] f32 + mask[4096,8192] bool,
shards rows across 8 NeuronCores (512 rows each), runs the whole 6-candidate
ALS search on-device with one tiny AllReduce for the global-MSE argmin, and
returns the FULL [4096,8192] f32 output.

Per-core layout: 512 rows = 4 row-groups of 128 partitions; the x shard
(16 MiB) stays resident in SBUF, so HBM traffic is one read + one write.
Each of the 6 candidate passes streams x through:
  A  (ACT):    t = relu(x*(1/s) - o/s)        per-partition scale/bias
  B1 (GPSIMD): t = (t + 2^23) - 2^23          exact round-half-even (t >= 0),
               applied in-place per 2048-wide half chunk
  B2 (DVE):    q = min(t, 3)  + fused row-sum -> sum(q)
  C  (DVE):    (x - o)*q      + fused row-sum -> numer
  D  (ACT):    q^2            + fused row-sum -> sum(q^2)
MSE is reconstructed algebraically from sum(x), sum(x^2) and the pass stats;
the 6 per-core partial MSEs are AllReduced once and the best (s, o) snapshot
is selected on-device.
"""

import numpy as np

R_TOTAL, C = 4096, 8192
N_CORES = 8
R = R_TOTAL // N_CORES          # 512 rows per core
G = R // 128                    # 4 row-groups of 128 partitions
FD = 4096                       # wide chunk (A/B1, DMA)
HF = 2048                       # half chunk (B2/C/D)
MAXQ = 3.0
EPS = 1e-8
N_PASS = 6                      # init + 5 ALS iterations = 6 (s,o) candidates
MAGIC = 8388608.0               # 2^23: add/sub rounds non-negative f32 (RNE)

_CACHE = {}


def _build_program(n_devices=N_CORES, with_collective=True):
    import concourse.bacc as bacc
    import concourse.bass as bass
    import concourse.mybir as mybir
    from concourse import tile

    f32 = mybir.dt.float32
    Alu = mybir.AluOpType
    Act = mybir.ActivationFunctionType
    X = mybir.AxisListType.X

    nc = bacc.Bacc("TRN2", target_bir_lowering=False, debug=False,
                   num_devices=n_devices)
    x_d = nc.dram_tensor("x", [R, C], f32, kind="ExternalInput").ap()
    out_d = nc.dram_tensor("out", [R, C], f32, kind="ExternalOutput").ap()

    with tile.TileContext(nc) as tc:
        with (
            tc.tile_pool(name="big", bufs=1) as big,
            tc.tile_pool(name="tp", bufs=2) as tp,
            tc.tile_pool(name="rp", bufs=2) as rp,
            tc.tile_pool(name="qp", bufs=3) as qp,
            tc.tile_pool(name="pc", bufs=1, space="PSUM") as pcp,
            tc.tile_pool(name="pd", bufs=1, space="PSUM") as pdp,
            tc.tile_pool(name="small", bufs=1) as small,
            tc.tile_pool(name="dram", bufs=1, space="DRAM") as dram,
        ):
            xs = big.tile([128, G * C], f32, tag="xs", name="xs")

            def xw(g, h2):  # wide chunk view of resident x [128, 4096]
                off = g * C + h2 * FD
                return xs[:, off:off + FD]

            def st(name, n):
                return small.tile([128, n], f32, tag=name, name=name)

            # chunk stat accumulators: wide (col=g*2+h2) and half (col=g*4+h)
            vminc, vmaxc, sxc = st("vminc", 8), st("vmaxc", 8), st("sxc", 8)
            sxxc = st("sxxc", 16)
            sqc, numc, sqqc = st("sqc", 16), st("numc", 16), st("sqqc", 16)
            sx, sxx = st("sx", 4), st("sxx", 4)
            s_snap, o_snap = st("s_snap", 24), st("o_snap", 24)
            msebuf = st("msebuf", 8)
            nc.vector.memset(msebuf[:], 0.0)

            # ---------- load + init stats ----------
            for g in range(G):
                for h2 in range(2):
                    c2 = g * 2 + h2
                    nc.sync.dma_start(
                        xw(g, h2),
                        x_d[g * 128:(g + 1) * 128, h2 * FD:(h2 + 1) * FD])
                    t = tp.tile([128, FD], f32, tag="t", name="t")
                    t2 = tp.tile([128, FD], f32, tag="t", name="t")
                    q0 = qp.tile([128, HF], f32, tag="q", name="q")
                    q1 = qp.tile([128, HF], f32, tag="q", name="q")
                    nc.vector.tensor_scalar(
                        out=t[:], in0=xw(g, h2), scalar1=3.4e38, scalar2=None,
                        op0=Alu.min, op1=Alu.min, accum_out=vminc[:, c2:c2 + 1])
                    nc.vector.tensor_scalar(
                        out=t2[:], in0=xw(g, h2), scalar1=-3.4e38, scalar2=None,
                        op0=Alu.max, op1=Alu.max, accum_out=vmaxc[:, c2:c2 + 1])
                    nc.vector.tensor_scalar(
                        out=q0[:], in0=xw(g, h2)[:, 0:HF], scalar1=-3.4e38,
                        scalar2=None, op0=Alu.max, op1=Alu.add,
                        accum_out=sxc[:, c2:c2 + 1])
                    # sum(x) second half: stash in the odd cols of sxxc
                    nc.vector.tensor_scalar(
                        out=q1[:], in0=xw(g, h2)[:, HF:FD], scalar1=-3.4e38,
                        scalar2=None, op0=Alu.max, op1=Alu.add,
                        accum_out=sxxc[:, c2 * 2 + 1:c2 * 2 + 2])
                    # sum(x^2) on ACT (PSUM out), two halves
                    pd = pdp.tile([128, HF], f32, tag="pd", name="pd")
                    nc.scalar.activation(out=pd[:], in_=xw(g, h2)[:, 0:HF],
                                         func=Act.Square,
                                         accum_out=sqqc[:, c2 * 2:c2 * 2 + 1])
                    pd2 = pdp.tile([128, HF], f32, tag="pd", name="pd")
                    nc.scalar.activation(out=pd2[:], in_=xw(g, h2)[:, HF:FD],
                                         func=Act.Square,
                                         accum_out=sqqc[:, c2 * 2 + 1:c2 * 2 + 2])

            def combine2(chunk_tile, out_tile, op):
                nc.vector.tensor_reduce(
                    out_tile[:], chunk_tile[:].rearrange("p (g h) -> p g h", g=G),
                    axis=X, op=op)

            def combine4(chunk_tile, out_tile):
                nc.vector.tensor_reduce(
                    out_tile[:], chunk_tile[:].rearrange("p (g h) -> p g h", g=G),
                    axis=X, op=Alu.add)

            vmin, vmax = st("vmin", 4), st("vmax", 4)
            combine2(vminc, vmin, Alu.min)
            combine2(vmaxc, vmax, Alu.max)
            # sum(x): first halves in sxc cols (wide), second halves were
            # stashed in odd cols of sxxc; sum(x^2) lives in even+odd sqqc.
            sxh = st("sxh", 4)
            combine2(sxc, sxh, Alu.add)
            sxh2 = st("sxh2", 4)
            nc.vector.tensor_reduce(
                sxh2[:], sxxc[:, 1:16:2].rearrange("p (g h) -> p g h", g=G),
                axis=X, op=Alu.add)
            nc.vector.tensor_add(sx[:], sxh[:], sxh2[:])
            combine4(sqqc, sxx)

            # ---------- init scale / offset ----------
            scur, ocur = st("scur", 4), st("ocur", 4)
            invs, nzp = st("invs", 4), st("nzp", 4)
            ta, tb, td = st("ta", 4), st("tb", 4), st("td", 4)
            eps_t = st("eps_t", 1)
            nc.vector.memset(eps_t[:], EPS)

            nc.vector.tensor_scalar(out=ta[:], in0=vmin[:], scalar1=0.0,
                                    scalar2=None, op0=Alu.min)   # xmin
            nc.vector.tensor_scalar(out=tb[:], in0=vmax[:], scalar1=0.0,
                                    scalar2=None, op0=Alu.max)   # xmax
            nc.vector.tensor_sub(tb[:], tb[:], ta[:])            # xmax-xmin
            nc.vector.tensor_scalar(out=scur[:], in0=tb[:], scalar1=1.0 / MAXQ,
                                    scalar2=EPS, op0=Alu.mult, op1=Alu.max)
            nc.vector.tensor_scalar(out=ta[:], in0=ta[:], scalar1=-1.0,
                                    scalar2=None, op0=Alu.mult)  # -xmin >= 0
            nc.vector.reciprocal(tb[:], scur[:])
            nc.vector.tensor_mul(td[:], ta[:], tb[:])
            nc.vector.tensor_scalar(out=td[:], in0=td[:], scalar1=MAGIC,
                                    scalar2=MAGIC, op0=Alu.add, op1=Alu.subtract)
            nc.vector.tensor_scalar(out=td[:], in0=td[:], scalar1=MAXQ,
                                    scalar2=None, op0=Alu.min)   # zp0 in [0,3]
            nc.vector.tensor_mul(ocur[:], scur[:], td[:])
            nc.vector.tensor_scalar(out=ocur[:], in0=ocur[:], scalar1=-1.0,
                                    scalar2=None, op0=Alu.mult)

            def set_params():
                nc.vector.reciprocal(invs[:], scur[:])
                nc.vector.tensor_mul(nzp[:], ocur[:], invs[:])
                nc.vector.tensor_scalar(out=nzp[:], in0=nzp[:], scalar1=-1.0,
                                        scalar2=None, op0=Alu.mult)

            def snapshot(k):
                nc.scalar.copy(s_snap[:, k * 4:(k + 1) * 4], scur[:])
                nc.scalar.copy(o_snap[:, k * 4:(k + 1) * 4], ocur[:])

            set_params()
            snapshot(0)

            sq_row, num_row = st("sq_row", 4), st("num_row", 4)
            sqq_row = st("sqq_row", 4)
            mrow, mr1 = st("mrow", 4), st("mr1", 1)
            u1, u2, u3 = st("u1", 4), st("u2", 4), st("u3", 4)
            mfl = st("mfl", 4)

            # ---------- 6 stats passes ----------
            for k in range(N_PASS):
                for g in range(G):
                    for h2 in range(2):
                        t = tp.tile([128, FD], f32, tag="t", name="t")
                        # A: t = relu(x*invs - zp)   [ACT, wide]
                        nc.scalar.activation(out=t[:], in_=xw(g, h2),
                                             func=Act.Relu,
                                             bias=nzp[:, g:g + 1],
                                             scale=invs[:, g:g + 1])
                        for hh in range(2):
                            thh = t[:, hh * HF:(hh + 1) * HF]
                            nc.gpsimd.tensor_scalar(out=thh, in0=thh,
                                                    scalar1=MAGIC, scalar2=MAGIC,
                                                    op0=Alu.add, op1=Alu.subtract)
                        for hh in range(2):
                            c = g * 4 + h2 * 2 + hh
                            th = t[:, hh * HF:(hh + 1) * HF]
                            xh = xw(g, h2)[:, hh * HF:(hh + 1) * HF]
                            q = qp.tile([128, HF], f32, tag="q", name="q")
                            pcx = pcp.tile([128, HF], f32, tag="pc", name="pc")
                            pd = pdp.tile([128, HF], f32, tag="pd", name="pd")
                            # B2: q = min(t,3) + accum sum(q)  [DVE]
                            nc.vector.tensor_scalar(
                                out=q[:], in0=th, scalar1=MAXQ, scalar2=None,
                                op0=Alu.min, op1=Alu.add,
                                accum_out=sqc[:, c:c + 1])
                            # C: (x-o)*q + accum -> numer  [DVE]
                            nc.vector.scalar_tensor_tensor(
                                out=pcx[:], in0=xh, scalar=ocur[:, g:g + 1],
                                in1=q[:], op0=Alu.subtract, op1=Alu.mult,
                                accum_out=numc[:, c:c + 1])
                            # D: q^2 + accum -> sum(q^2)  [ACT]
                            nc.scalar.activation(
                                out=pd[:], in_=q[:], func=Act.Square,
                                accum_out=sqqc[:, c:c + 1])

                combine4(sqc, sq_row)
                combine4(numc, num_row)
                combine4(sqqc, sqq_row)

                # mse_k (this pass's s,o):
                #   sxx - 2*o*sx + o^2*N - 2*s*num + s^2*sqq
                nc.vector.tensor_mul(u1[:], ocur[:], sx[:])
                nc.vector.tensor_scalar(out=u1[:], in0=u1[:], scalar1=-2.0,
                                        scalar2=None, op0=Alu.mult)
                nc.vector.tensor_mul(u2[:], ocur[:], ocur[:])
                nc.vector.tensor_scalar(out=u2[:], in0=u2[:], scalar1=float(C),
                                        scalar2=None, op0=Alu.mult)
                nc.vector.tensor_add(u1[:], u1[:], u2[:])
                nc.vector.tensor_mul(u2[:], scur[:], num_row[:])
                nc.vector.tensor_scalar(out=u2[:], in0=u2[:], scalar1=-2.0,
                                        scalar2=None, op0=Alu.mult)
                nc.vector.tensor_mul(u3[:], scur[:], scur[:])
                nc.vector.tensor_mul(u3[:], u3[:], sqq_row[:])
                nc.vector.tensor_add(u2[:], u2[:], u3[:])
                nc.vector.tensor_add(u1[:], u1[:], u2[:])
                nc.vector.tensor_add(mrow[:], u1[:], sxx[:])
                nc.vector.tensor_reduce(mr1[:], mrow[:], axis=X, op=Alu.add)
                nc.gpsimd.partition_all_reduce(
                    msebuf[:, k:k + 1], mr1[:], channels=128,
                    reduce_op=bass.bass_isa.ReduceOp.add)

                if k < N_PASS - 1:
                    # ALS update
                    nc.vector.tensor_scalar(out=mfl[:], in0=sqq_row[:],
                                            scalar1=eps_t[:], scalar2=None,
                                            op0=Alu.is_le)
                    nc.vector.tensor_scalar(out=u1[:], in0=sqq_row[:],
                                            scalar1=EPS, scalar2=None,
                                            op0=Alu.add)
                    nc.vector.reciprocal(u2[:], u1[:])
                    nc.vector.tensor_mul(u2[:], num_row[:], u2[:])
                    # keep old scale where denom <= EPS
                    nc.vector.tensor_sub(u3[:], scur[:], u2[:])
                    nc.vector.tensor_mul(u3[:], mfl[:], u3[:])
                    nc.vector.tensor_add(u3[:], u2[:], u3[:])
                    # scur = max(|u3|, EPS)
                    nc.vector.tensor_scalar(out=u1[:], in0=u3[:], scalar1=-1.0,
                                            scalar2=None, op0=Alu.mult)
                    nc.vector.tensor_max(u1[:], u3[:], u1[:])
                    nc.vector.tensor_scalar(out=scur[:], in0=u1[:], scalar1=EPS,
                                            scalar2=None, op0=Alu.max)
                    # ocur = (sx - scur*sq) / N
                    nc.vector.tensor_mul(u1[:], scur[:], sq_row[:])
                    nc.vector.tensor_sub(u2[:], sx[:], u1[:])
                    nc.vector.tensor_scalar(out=ocur[:], in0=u2[:],
                                            scalar1=1.0 / float(C),
                                            scalar2=None, op0=Alu.mult)
                    set_params()
                    snapshot(k + 1)

            # ---------- global argmin over 6 candidates ----------
            glob = st("glob", 8)
            if with_collective:
                cc_in = dram.tile([1, 8], f32, tag="cc_in", name="cc_in")
                cc_out = dram.tile([1, 8], f32, tag="cc_out", name="cc_out")
                nc.sync.dma_start(cc_in[:], msebuf[0:1, :])
                nc.gpsimd.collective_compute(
                    "AllReduce", Alu.add, replica_groups=[list(range(N_CORES))],
                    ins=[cc_in.opt()], outs=[cc_out.opt()])
                nc.sync.dma_start(glob[0:1, :], cc_out[:])
                nc.gpsimd.partition_broadcast(glob[:, :], glob[0:1, :],
                                              channels=128)
            else:
                nc.vector.tensor_copy(glob[:], msebuf[:])

            bmin, mk = st("bmin", 1), st("mk", 1)
            sbest, obest = st("sbest", 4), st("obest", 4)
            sd = st("sd", 4)
            nc.vector.tensor_reduce(bmin[:], glob[:, 0:6], axis=X, op=Alu.min)
            nc.scalar.copy(sbest[:], s_snap[:, 0:4])
            nc.scalar.copy(obest[:], o_snap[:, 0:4])
            # first-wins argmin: blend from k=5 down to k=0
            for k in range(N_PASS - 1, -1, -1):
                nc.vector.tensor_scalar(out=mk[:], in0=glob[:, k:k + 1],
                                        scalar1=bmin[:], scalar2=None,
                                        op0=Alu.is_le)
                for snap, best in ((s_snap, sbest), (o_snap, obest)):
                    nc.vector.tensor_sub(sd[:], snap[:, k * 4:(k + 1) * 4],
                                         best[:])
                    nc.vector.tensor_scalar(out=sd[:], in0=sd[:],
                                            scalar1=mk[:], scalar2=None,
                                            op0=Alu.mult)
                    nc.vector.tensor_add(best[:], best[:], sd[:])
            invsb, nzpb = st("invsb", 4), st("nzpb", 4)
            nc.vector.reciprocal(invsb[:], sbest[:])
            nc.vector.tensor_mul(nzpb[:], obest[:], invsb[:])
            nc.vector.tensor_scalar(out=nzpb[:], in0=nzpb[:], scalar1=-1.0,
                                    scalar2=None, op0=Alu.mult)

            # ---------- final pass: quantize + reconstruct + store ----------
            for g in range(G):
                for h2 in range(2):
                    t = tp.tile([128, FD], f32, tag="t", name="t")
                    nc.scalar.activation(out=t[:], in_=xw(g, h2), func=Act.Relu,
                                         bias=nzpb[:, g:g + 1],
                                         scale=invsb[:, g:g + 1])
                    nc.gpsimd.tensor_scalar(out=t[:], in0=t[:], scalar1=MAGIC,
                                            scalar2=MAGIC, op0=Alu.add,
                                            op1=Alu.subtract)
                    for hh in range(2):
                        th = t[:, hh * HF:(hh + 1) * HF]
                        q = qp.tile([128, HF], f32, tag="q", name="q")
                        rec = rp.tile([128, HF], f32, tag="r", name="rec")
                        nc.vector.tensor_scalar(out=q[:], in0=th, scalar1=MAXQ,
                                                scalar2=None, op0=Alu.min)
                        nc.vector.tensor_scalar(out=rec[:], in0=q[:],
                                                scalar1=sbest[:, g:g + 1],
                                                scalar2=obest[:, g:g + 1],
                                                op0=Alu.mult, op1=Alu.add)
                        nc.sync.dma_start(
                            out_d[g * 128:(g + 1) * 128,
                                  h2 * FD + hh * HF:h2 * FD + (hh + 1) * HF],
                            rec[:])

    nc.compile()
    return nc


def _get_program():
    if "nc" not in _CACHE:
        _CACHE["nc"] = _build_program()
    return _CACHE["nc"]


def _numpy_fallback(w, mask):
    # Exact mirror of the reference (only used if mask isn't all-ones).
    x = w.astype(np.float32)
    mask_f = mask.astype(np.float32)
    valid = mask_f.sum(axis=1)
    has_valid = valid > 0
    big = np.float32(np.inf)
    xmin = np.where(mask, x, big).min(axis=1)
    xmax = np.where(mask, x, -big).max(axis=1)
    xmin = np.where(has_valid, xmin, -1.0).astype(np.float32)
    xmax = np.where(has_valid, xmax, 1.0).astype(np.float32)
    xmin = np.minimum(xmin, 0.0)
    xmax = np.maximum(xmax, 0.0)
    scale = np.maximum((xmax - xmin) / np.float32(MAXQ), np.float32(EPS))
    zp = np.clip(np.round(-xmin / scale), 0.0, MAXQ).astype(np.float32)
    offset = -scale * zp
    total_valid = max(valid.sum(), 1.0)

    def masked_mse(s, o):
        q = np.clip(np.round((x - o[:, None]) / s[:, None]), 0.0, MAXQ) * mask_f
        recon = s[:, None] * q + o[:, None]
        return (((x - recon) ** 2) * mask_f).sum() / total_valid

    best_mse = masked_mse(scale, offset)
    best_s, best_o = scale.copy(), offset.copy()
    for _ in range(5):
        q = np.clip(np.round((x - offset[:, None]) / scale[:, None]), 0.0, MAXQ) * mask_f
        denom = (q * q).sum(axis=1)
        numer = ((x - offset[:, None]) * q).sum(axis=1)
        new_s = np.where(denom > EPS, numer / (denom + EPS), scale)
        new_s = np.maximum(np.abs(new_s), EPS).astype(np.float32)
        new_o = ((x - new_s[:, None] * q) * mask_f).sum(axis=1) / np.maximum(valid, 1.0)
        scale = np.where(has_valid, new_s, scale).astype(np.float32)
        offset = np.where(has_valid, new_o, offset).astype(np.float32)
        cur = masked_mse(scale, offset)
        if cur < best_mse:
            best_mse, best_s, best_o = cur, scale.copy(), offset.copy()
    scale, offset = best_s, best_o
    q = np.clip(np.round((x - offset[:, None]) / scale[:, None]), 0.0, MAXQ) * mask_f
    x_hat = scale[:, None] * q + offset[:, None]
    return np.where(mask, x_hat, x).astype(np.float32)


def kernel(w, mask):
    w = np.ascontiguousarray(np.asarray(w, dtype=np.float32))
    mask_np = np.asarray(mask)
    if not mask_np.all():
        return _numpy_fallback(w, mask_np)

    from concourse.bass_utils import run_bass_kernel_spmd
    nc = _get_program()
    in_maps = [{"x": np.ascontiguousarray(w[i * R:(i + 1) * R])}
               for i in range(N_CORES)]
    res = run_bass_kernel_spmd(nc, in_maps, list(range(N_CORES))).results
    return np.concatenate([res[i]["out"] for i in range(N_CORES)], axis=0)


# revision 38
# speedup vs baseline: 1.0096x; 1.0096x over previous
"""Trainium2 kernel for per-row iterative 2-bit affine quantization (ALS).

Self-contained: accepts FULL inputs w[4096,8192] f32 + mask[4096,8192] bool,
shards rows across 8 NeuronCores (512 rows each), runs the whole 6-candidate
ALS search on-device with one tiny AllReduce for the global-MSE argmin, and
returns the FULL [4096,8192] f32 output.

Per-core layout: 512 rows = 4 row-groups of 128 partitions; the x shard
(16 MiB) stays resident in SBUF, so HBM traffic is one read + one write.
Each of the 6 candidate passes streams x through:
  A  (ACT):    t = relu(x*(1/s) - o/s)        per-partition scale/bias
  B1 (GPSIMD): t = (t + 2^23) - 2^23          exact round-half-even (t >= 0),
               applied in-place per 2048-wide half chunk
  B2 (DVE):    q = min(t, 3)  + fused row-sum -> sum(q)
  C  (DVE):    (x - o)*q      + fused row-sum -> numer
  D  (ACT):    q^2            + fused row-sum -> sum(q^2)
MSE is reconstructed algebraically from sum(x), sum(x^2) and the pass stats;
the 6 per-core partial MSEs are AllReduced once and the best (s, o) snapshot
is selected on-device.
"""

import numpy as np

R_TOTAL, C = 4096, 8192
N_CORES = 8
R = R_TOTAL // N_CORES          # 512 rows per core
G = R // 128                    # 4 row-groups of 128 partitions
FD = 4096                       # wide chunk (A/B1, DMA)
HF = 2048                       # half chunk (B2/C/D)
MAXQ = 3.0
EPS = 1e-8
N_PASS = 6                      # init + 5 ALS iterations = 6 (s,o) candidates
MAGIC = 8388608.0               # 2^23: add/sub rounds non-negative f32 (RNE)

_CACHE = {}


def _build_program(n_devices=N_CORES, with_collective=True):
    import concourse.bacc as bacc
    import concourse.bass as bass
    import concourse.mybir as mybir
    from concourse import tile

    f32 = mybir.dt.float32
    Alu = mybir.AluOpType
    Act = mybir.ActivationFunctionType
    X = mybir.AxisListType.X

    nc = bacc.Bacc("TRN2", target_bir_lowering=False, debug=False,
                   num_devices=n_devices)
    x_d = nc.dram_tensor("x", [R, C], f32, kind="ExternalInput").ap()
    out_d = nc.dram_tensor("out", [R, C], f32, kind="ExternalOutput").ap()

    with tile.TileContext(nc) as tc:
        with (
            tc.tile_pool(name="big", bufs=1) as big,
            tc.tile_pool(name="tp", bufs=2) as tp,
            tc.tile_pool(name="rp", bufs=2) as rp,
            tc.tile_pool(name="qp", bufs=3) as qp,
            tc.tile_pool(name="pc", bufs=1, space="PSUM") as pcp,
            tc.tile_pool(name="pd", bufs=1, space="PSUM") as pdp,
            tc.tile_pool(name="small", bufs=1) as small,
            tc.tile_pool(name="dram", bufs=1, space="DRAM") as dram,
        ):
            xs = big.tile([128, G * C], f32, tag="xs", name="xs")

            def xw(g, h2):  # wide chunk view of resident x [128, 4096]
                off = g * C + h2 * FD
                return xs[:, off:off + FD]

            def st(name, n):
                return small.tile([128, n], f32, tag=name, name=name)

            # chunk stat accumulators: wide (col=g*2+h2) and half (col=g*4+h)
            vminc, vmaxc, sxc = st("vminc", 8), st("vmaxc", 8), st("sxc", 8)
            sxxc = st("sxxc", 16)
            sqc, numc, sqqc = st("sqc", 16), st("numc", 16), st("sqqc", 16)
            sx, sxx = st("sx", 4), st("sxx", 4)
            s_snap, o_snap = st("s_snap", 24), st("o_snap", 24)
            msebuf = st("msebuf", 8)
            nc.vector.memset(msebuf[:], 0.0)

            # ---------- load + init stats ----------
            for g in range(G):
                for h2 in range(2):
                    c2 = g * 2 + h2
                    nc.sync.dma_start(
                        xw(g, h2),
                        x_d[g * 128:(g + 1) * 128, h2 * FD:(h2 + 1) * FD])
                    t = tp.tile([128, FD], f32, tag="t", name="t")
                    t2 = tp.tile([128, FD], f32, tag="t", name="t")
                    q0 = qp.tile([128, HF], f32, tag="q", name="q")
                    q1 = qp.tile([128, HF], f32, tag="q", name="q")
                    nc.vector.tensor_scalar(
                        out=t[:], in0=xw(g, h2), scalar1=3.4e38, scalar2=None,
                        op0=Alu.min, op1=Alu.min, accum_out=vminc[:, c2:c2 + 1])
                    nc.vector.tensor_scalar(
                        out=t2[:], in0=xw(g, h2), scalar1=-3.4e38, scalar2=None,
                        op0=Alu.max, op1=Alu.max, accum_out=vmaxc[:, c2:c2 + 1])
                    nc.vector.tensor_scalar(
                        out=q0[:], in0=xw(g, h2)[:, 0:HF], scalar1=-3.4e38,
                        scalar2=None, op0=Alu.max, op1=Alu.add,
                        accum_out=sxc[:, c2:c2 + 1])
                    # sum(x) second half: stash in the odd cols of sxxc
                    nc.vector.tensor_scalar(
                        out=q1[:], in0=xw(g, h2)[:, HF:FD], scalar1=-3.4e38,
                        scalar2=None, op0=Alu.max, op1=Alu.add,
                        accum_out=sxxc[:, c2 * 2 + 1:c2 * 2 + 2])
                    # sum(x^2) on ACT (PSUM out), two halves
                    pd = pdp.tile([128, HF], f32, tag="pd", name="pd")
                    nc.scalar.activation(out=pd[:], in_=xw(g, h2)[:, 0:HF],
                                         func=Act.Square,
                                         accum_out=sqqc[:, c2 * 2:c2 * 2 + 1])
                    pd2 = pdp.tile([128, HF], f32, tag="pd", name="pd")
                    nc.scalar.activation(out=pd2[:], in_=xw(g, h2)[:, HF:FD],
                                         func=Act.Square,
                                         accum_out=sqqc[:, c2 * 2 + 1:c2 * 2 + 2])

            def combine2(chunk_tile, out_tile, op):
                nc.vector.tensor_reduce(
                    out_tile[:], chunk_tile[:].rearrange("p (g h) -> p g h", g=G),
                    axis=X, op=op)

            def combine4(chunk_tile, out_tile):
                nc.vector.tensor_reduce(
                    out_tile[:], chunk_tile[:].rearrange("p (g h) -> p g h", g=G),
                    axis=X, op=Alu.add)

            vmin, vmax = st("vmin", 4), st("vmax", 4)
            combine2(vminc, vmin, Alu.min)
            combine2(vmaxc, vmax, Alu.max)
            # sum(x): first halves in sxc cols (wide), second halves were
            # stashed in odd cols of sxxc; sum(x^2) lives in even+odd sqqc.
            sxh = st("sxh", 4)
            combine2(sxc, sxh, Alu.add)
            sxh2 = st("sxh2", 4)
            nc.vector.tensor_reduce(
                sxh2[:], sxxc[:, 1:16:2].rearrange("p (g h) -> p g h", g=G),
                axis=X, op=Alu.add)
            nc.vector.tensor_add(sx[:], sxh[:], sxh2[:])
            sxN = st("sxN", 4)
            nc.vector.tensor_scalar(out=sxN[:], in0=sx[:],
                                    scalar1=1.0 / float(C), scalar2=None,
                                    op0=Alu.mult)
            combine4(sqqc, sxx)

            # ---------- init scale / offset ----------
            scur, ocur = st("scur", 4), st("ocur", 4)
            invs, nzp = st("invs", 4), st("nzp", 4)
            ta, tb, td = st("ta", 4), st("tb", 4), st("td", 4)
            eps_t = st("eps_t", 1)
            nc.vector.memset(eps_t[:], EPS)

            nc.vector.tensor_scalar(out=ta[:], in0=vmin[:], scalar1=0.0,
                                    scalar2=None, op0=Alu.min)   # xmin
            nc.vector.tensor_scalar(out=tb[:], in0=vmax[:], scalar1=0.0,
                                    scalar2=None, op0=Alu.max)   # xmax
            nc.vector.tensor_sub(tb[:], tb[:], ta[:])            # xmax-xmin
            nc.vector.tensor_scalar(out=scur[:], in0=tb[:], scalar1=1.0 / MAXQ,
                                    scalar2=EPS, op0=Alu.mult, op1=Alu.max)
            nc.vector.tensor_scalar(out=ta[:], in0=ta[:], scalar1=-1.0,
                                    scalar2=None, op0=Alu.mult)  # -xmin >= 0
            nc.vector.reciprocal(tb[:], scur[:])
            nc.vector.tensor_mul(td[:], ta[:], tb[:])
            nc.vector.tensor_scalar(out=td[:], in0=td[:], scalar1=MAGIC,
                                    scalar2=MAGIC, op0=Alu.add, op1=Alu.subtract)
            nc.vector.tensor_scalar(out=td[:], in0=td[:], scalar1=MAXQ,
                                    scalar2=None, op0=Alu.min)   # zp0 in [0,3]
            nc.vector.tensor_mul(ocur[:], scur[:], td[:])
            nc.vector.tensor_scalar(out=ocur[:], in0=ocur[:], scalar1=-1.0,
                                    scalar2=None, op0=Alu.mult)

            def set_params():
                nc.vector.reciprocal(invs[:], scur[:])
                nc.vector.tensor_mul(nzp[:], ocur[:], invs[:])
                nc.vector.tensor_scalar(out=nzp[:], in0=nzp[:], scalar1=-1.0,
                                        scalar2=None, op0=Alu.mult)

            def snapshot(k):
                nc.scalar.copy(s_snap[:, k * 4:(k + 1) * 4], scur[:])
                nc.scalar.copy(o_snap[:, k * 4:(k + 1) * 4], ocur[:])

            set_params()
            snapshot(0)

            sq_row, num_row = st("sq_row", 4), st("num_row", 4)
            sqq_row = st("sqq_row", 4)
            mrow, mr1 = st("mrow", 4), st("mr1", 1)
            u1, u2, u3 = st("u1", 4), st("u2", 4), st("u3", 4)
            v1, v2, v3 = st("v1", 4), st("v2", 4), st("v3", 4)
            mfl = st("mfl", 4)

            # ---------- 6 stats passes ----------
            for k in range(N_PASS):
                for g in range(G):
                    for h2 in range(2):
                        t = tp.tile([128, FD], f32, tag="t", name="t")
                        # A: t = relu(x*invs - zp)   [ACT, wide]
                        nc.scalar.activation(out=t[:], in_=xw(g, h2),
                                             func=Act.Relu,
                                             bias=nzp[:, g:g + 1],
                                             scale=invs[:, g:g + 1])
                        for hh in range(2):
                            thh = t[:, hh * HF:(hh + 1) * HF]
                            nc.gpsimd.tensor_scalar(out=thh, in0=thh,
                                                    scalar1=MAGIC, scalar2=MAGIC,
                                                    op0=Alu.add, op1=Alu.subtract)
                        for hh in range(2):
                            c = g * 4 + h2 * 2 + hh
                            th = t[:, hh * HF:(hh + 1) * HF]
                            xh = xw(g, h2)[:, hh * HF:(hh + 1) * HF]
                            q = qp.tile([128, HF], f32, tag="q", name="q")
                            pcx = pcp.tile([128, HF], f32, tag="pc", name="pc")
                            pd = pdp.tile([128, HF], f32, tag="pd", name="pd")
                            # B2: q = min(t,3) + accum sum(q)  [DVE]
                            nc.vector.tensor_scalar(
                                out=q[:], in0=th, scalar1=MAXQ, scalar2=None,
                                op0=Alu.min, op1=Alu.add,
                                accum_out=sqc[:, c:c + 1])
                            # C: (x-o)*q + accum -> numer  [DVE]
                            nc.vector.scalar_tensor_tensor(
                                out=pcx[:], in0=xh, scalar=ocur[:, g:g + 1],
                                in1=q[:], op0=Alu.subtract, op1=Alu.mult,
                                accum_out=numc[:, c:c + 1])
                            # D: q^2 + accum -> sum(q^2)  [ACT]
                            nc.scalar.activation(
                                out=pd[:], in_=q[:], func=Act.Square,
                                accum_out=sqqc[:, c:c + 1])

                combine4(sqc, sq_row)
                combine4(numc, num_row)
                combine4(sqqc, sqq_row)

                # mse_k (this pass's s,o), computed on GPSIMD off the
                # DVE critical path:
                #   sxx - 2*o*sx + o^2*N - 2*s*num + s^2*sqq
                nc.gpsimd.tensor_mul(v1[:], ocur[:], sx[:])
                nc.gpsimd.tensor_scalar(out=v1[:], in0=v1[:], scalar1=-2.0,
                                        scalar2=None, op0=Alu.mult)
                nc.gpsimd.tensor_mul(v2[:], ocur[:], ocur[:])
                nc.gpsimd.tensor_scalar(out=v2[:], in0=v2[:], scalar1=float(C),
                                        scalar2=None, op0=Alu.mult)
                nc.gpsimd.tensor_add(v1[:], v1[:], v2[:])
                nc.gpsimd.tensor_mul(v2[:], scur[:], num_row[:])
                nc.gpsimd.tensor_scalar(out=v2[:], in0=v2[:], scalar1=-2.0,
                                        scalar2=None, op0=Alu.mult)
                nc.gpsimd.tensor_mul(v3[:], scur[:], scur[:])
                nc.gpsimd.tensor_mul(v3[:], v3[:], sqq_row[:])
                nc.gpsimd.tensor_add(v2[:], v2[:], v3[:])
                nc.gpsimd.tensor_add(v1[:], v1[:], v2[:])
                nc.gpsimd.tensor_add(mrow[:], v1[:], sxx[:])
                nc.vector.tensor_reduce(mr1[:], mrow[:], axis=X, op=Alu.add)
                nc.gpsimd.partition_all_reduce(
                    msebuf[:, k:k + 1], mr1[:], channels=128,
                    reduce_op=bass.bass_isa.ReduceOp.add)

                if k < N_PASS - 1:
                    # ALS update
                    nc.vector.tensor_scalar(out=mfl[:], in0=sqq_row[:],
                                            scalar1=eps_t[:], scalar2=None,
                                            op0=Alu.is_le)
                    nc.vector.tensor_scalar(out=u1[:], in0=sqq_row[:],
                                            scalar1=EPS, scalar2=None,
                                            op0=Alu.add)
                    nc.vector.reciprocal(u2[:], u1[:])
                    nc.vector.tensor_mul(u2[:], num_row[:], u2[:])
                    # keep old scale where denom <= EPS
                    nc.vector.tensor_sub(u3[:], scur[:], u2[:])
                    nc.vector.tensor_mul(u3[:], mfl[:], u3[:])
                    nc.vector.tensor_add(u3[:], u2[:], u3[:])
                    # scur = max(|u3|, EPS) in two ops
                    nc.vector.tensor_scalar(out=u1[:], in0=u3[:], scalar1=-1.0,
                                            scalar2=EPS, op0=Alu.mult,
                                            op1=Alu.max)
                    nc.vector.tensor_max(scur[:], u3[:], u1[:])
                    # ocur = sx/N - (scur*sq)/N  (fused)
                    nc.vector.tensor_mul(u1[:], scur[:], sq_row[:])
                    nc.vector.scalar_tensor_tensor(
                        out=ocur[:], in0=u1[:], scalar=-1.0 / float(C),
                        in1=sxN[:], op0=Alu.mult, op1=Alu.add)
                    set_params()
                    snapshot(k + 1)

            # ---------- global argmin over 6 candidates ----------
            glob = st("glob", 8)
            if with_collective:
                groups = [list(range(N_CORES))]
                # mse_0..4 are ready one pass early: their AllReduce overlaps
                # pass 5; only the [1,1] reduce of mse_5 sits on the tail.
                cc_in = dram.tile([1, 8], f32, tag="cc_in", name="cc_in")
                cc_out = dram.tile([1, 8], f32, tag="cc_out", name="cc_out")
                nc.sync.dma_start(cc_in[0:1, 0:5], msebuf[0:1, 0:5])
                nc.gpsimd.collective_compute(
                    "AllReduce", Alu.add, replica_groups=groups,
                    ins=[cc_in[0:1, 0:5]], outs=[cc_out[0:1, 0:5]])
                cc_in5 = dram.tile([1, 8], f32, tag="cc_in5", name="cc_in5")
                cc_out5 = dram.tile([1, 8], f32, tag="cc_out5", name="cc_out5")
                nc.sync.dma_start(cc_in5[0:1, 0:1], msebuf[0:1, 5:6])
                nc.gpsimd.collective_compute(
                    "AllReduce", Alu.add, replica_groups=groups,
                    ins=[cc_in5[0:1, 0:1]], outs=[cc_out5[0:1, 0:1]])
                nc.sync.dma_start(glob[0:1, 0:5], cc_out[0:1, 0:5])
                nc.sync.dma_start(glob[0:1, 5:6], cc_out5[0:1, 0:1])
                nc.gpsimd.partition_broadcast(glob[:, :], glob[0:1, :],
                                              channels=128)
            else:
                nc.vector.tensor_copy(glob[:], msebuf[:])

            bmin, mk = st("bmin", 1), st("mk", 1)
            sbest, obest = st("sbest", 4), st("obest", 4)
            sd = st("sd", 4)
            nc.vector.tensor_reduce(bmin[:], glob[:, 0:6], axis=X, op=Alu.min)
            nc.scalar.copy(sbest[:], s_snap[:, 0:4])
            nc.scalar.copy(obest[:], o_snap[:, 0:4])
            # first-wins argmin: blend from k=5 down to k=0
            for k in range(N_PASS - 1, -1, -1):
                nc.vector.tensor_scalar(out=mk[:], in0=glob[:, k:k + 1],
                                        scalar1=bmin[:], scalar2=None,
                                        op0=Alu.is_le)
                for snap, best in ((s_snap, sbest), (o_snap, obest)):
                    nc.vector.tensor_sub(sd[:], snap[:, k * 4:(k + 1) * 4],
                                         best[:])
                    nc.vector.tensor_scalar(out=sd[:], in0=sd[:],
                                            scalar1=mk[:], scalar2=None,
                                            op0=Alu.mult)
                    nc.vector.tensor_add(best[:], best[:], sd[:])
            invsb, nzpb = st("invsb", 4), st("nzpb", 4)
            nc.vector.reciprocal(invsb[:], sbest[:])
            nc.vector.tensor_mul(nzpb[:], obest[:], invsb[:])
            nc.vector.tensor_scalar(out=nzpb[:], in0=nzpb[:], scalar1=-1.0,
                                    scalar2=None, op0=Alu.mult)

            # ---------- final pass: quantize + reconstruct + store ----------
            for g in range(G):
                for h2 in range(2):
                    t = tp.tile([128, FD], f32, tag="t", name="t")
                    nc.scalar.activation(out=t[:], in_=xw(g, h2), func=Act.Relu,
                                         bias=nzpb[:, g:g + 1],
                                         scale=invsb[:, g:g + 1])
                    nc.gpsimd.tensor_scalar(out=t[:], in0=t[:], scalar1=MAGIC,
                                            scalar2=MAGIC, op0=Alu.add,
                                            op1=Alu.subtract)
                    for hh in range(2):
                        th = t[:, hh * HF:(hh + 1) * HF]
                        q = qp.tile([128, HF], f32, tag="q", name="q")
                        rec = rp.tile([128, HF], f32, tag="r", name="rec")
                        nc.vector.tensor_scalar(out=q[:], in0=th, scalar1=MAXQ,
                                                scalar2=None, op0=Alu.min)
                        nc.vector.tensor_scalar(out=rec[:], in0=q[:],
                                                scalar1=sbest[:, g:g + 1],
                                                scalar2=obest[:, g:g + 1],
                                                op0=Alu.mult, op1=Alu.add)
                        nc.sync.dma_start(
                            out_d[g * 128:(g + 1) * 128,
                                  h2 * FD + hh * HF:h2 * FD + (hh + 1) * HF],
                            rec[:])

    nc.compile()
    return nc


def _get_program():
    if "nc" not in _CACHE:
        _CACHE["nc"] = _build_program()
    return _CACHE["nc"]


def _numpy_fallback(w, mask):
    # Exact mirror of the reference (only used if mask isn't all-ones).
    x = w.astype(np.float32)
    mask_f = mask.astype(np.float32)
    valid = mask_f.sum(axis=1)
    has_valid = valid > 0
    big = np.float32(np.inf)
    xmin = np.where(mask, x, big).min(axis=1)
    xmax = np.where(mask, x, -big).max(axis=1)
    xmin = np.where(has_valid, xmin, -1.0).astype(np.float32)
    xmax = np.where(has_valid, xmax, 1.0).astype(np.float32)
    xmin = np.minimum(xmin, 0.0)
    xmax = np.maximum(xmax, 0.0)
    scale = np.maximum((xmax - xmin) / np.float32(MAXQ), np.float32(EPS))
    zp = np.clip(np.round(-xmin / scale), 0.0, MAXQ).astype(np.float32)
    offset = -scale * zp
    total_valid = max(valid.sum(), 1.0)

    def masked_mse(s, o):
        q = np.clip(np.round((x - o[:, None]) / s[:, None]), 0.0, MAXQ) * mask_f
        recon = s[:, None] * q + o[:, None]
        return (((x - recon) ** 2) * mask_f).sum() / total_valid

    best_mse = masked_mse(scale, offset)
    best_s, best_o = scale.copy(), offset.copy()
    for _ in range(5):
        q = np.clip(np.round((x - offset[:, None]) / scale[:, None]), 0.0, MAXQ) * mask_f
        denom = (q * q).sum(axis=1)
        numer = ((x - offset[:, None]) * q).sum(axis=1)
        new_s = np.where(denom > EPS, numer / (denom + EPS), scale)
        new_s = np.maximum(np.abs(new_s), EPS).astype(np.float32)
        new_o = ((x - new_s[:, None] * q) * mask_f).sum(axis=1) / np.maximum(valid, 1.0)
        scale = np.where(has_valid, new_s, scale).astype(np.float32)
        offset = np.where(has_valid, new_o, offset).astype(np.float32)
        cur = masked_mse(scale, offset)
        if cur < best_mse:
            best_mse, best_s, best_o = cur, scale.copy(), offset.copy()
    scale, offset = best_s, best_o
    q = np.clip(np.round((x - offset[:, None]) / scale[:, None]), 0.0, MAXQ) * mask_f
    x_hat = scale[:, None] * q + offset[:, None]
    return np.where(mask, x_hat, x).astype(np.float32)


def kernel(w, mask):
    w = np.ascontiguousarray(np.asarray(w, dtype=np.float32))
    mask_np = np.asarray(mask)
    if not mask_np.all():
        return _numpy_fallback(w, mask_np)

    from concourse.bass_utils import run_bass_kernel_spmd
    nc = _get_program()
    in_maps = [{"x": np.ascontiguousarray(w[i * R:(i + 1) * R])}
               for i in range(N_CORES)]
    res = run_bass_kernel_spmd(nc, in_maps, list(range(N_CORES))).results
    return np.concatenate([res[i]["out"] for i in range(N_CORES)], axis=0)
